# revision 5
# baseline (speedup 1.0000x reference)
"""Deductron kernel for Trainium2, 8 NeuronCores, time-sharded.

Math (matching the reference):
    h = sigmoid(W1 @ x + B1); left, right = h[:128], h[128:]
    a_t = left_t * right_t; b_t = 1 - left_t
    u_0 = 0; u_t = a_{t-1} * u_{t-1} + b_{t-1}   (z[:, t] = u_t)
    out = 1 - sigmoid(W2 @ z + B2) = sigmoid(-(W2 @ z + B2))

Sharding: the 65536-frame time axis is split into 8 chunks of 8192 plus a
512-frame washout halo per core (prod of 512 consecutive a's underflows, so
no cross-core state exchange is needed; core 0 zero-pads and bscale zeroes
the halo b's so its state stays exactly 0).

Perf structure (vs the naive per-512 pipeline):
  - x and W1 are host-quantized to fp8 e4m3 (W1 pre-scaled by 64, descaled in
    the activation's free affine). Halves input DMA; matmul runs at bf16 rate.
  - x is host-packed [128, nt, 4, tw] so one dma_start per supertile moves
    contiguous 4KB-per-partition chunks.
  - 1024-col supertiles: activations read 2 PSUM banks per instruction
    (FD=1024) to amortize the ~312-cycle ScalarE bubble; GEMM2 outputs are
    paired per half so the output activations are also FD=1024.
  - PSUM: psA (GEMM1 g0/g1, 2 bufs x 2 banks) + psB (GEMM2 o0/o1 pairs,
    2 bufs x 2 banks) = 8 banks.
"""

import sys

for _p in ("/opt/trn_rl_repo", "/opt/pypackages"):
    if _p not in sys.path:
        sys.path.append(_p)

import numpy as np
import ml_dtypes

# Problem constants (hardcoded per contract).
INPUT_LEN = 512
N_MEM = 128  # memory dim (recurrence state width) = one partition tile
OUT_LEN = 256
T_TOTAL = 65536
N_CORES = 8
T_LOC = T_TOTAL // N_CORES  # 8192 owned frames per core
HALO = 512                  # washout halo
TW = 512                    # base column tile (one PSUM bank of fp32)
ST = 1024                   # supertile: 2 banks per activation
W_IN = HALO + T_LOC         # per-core input width (8704)
NT = W_IN // TW             # 17 column tiles (tile 0 is pure halo)
NST = T_LOC // ST           # 8 supertiles over the owned region

F16_NP = np.float16
F8_NP = ml_dtypes.float8_e4m3
W1_SCALE = 64.0


def _build_nc(t_loc=T_LOC, halo=HALO, tw=TW):
    import concourse.tile as tile
    from concourse import bacc, mybir
    from contextlib import ExitStack

    F32 = mybir.dt.float32
    F16 = mybir.dt.float16
    F8 = mybir.dt.float8e4
    SIG = mybir.ActivationFunctionType.Sigmoid
    MUL = mybir.AluOpType.mult
    ADD = mybir.AluOpType.add

    w_in = halo + t_loc
    assert w_in % tw == 0 and halo == tw and t_loc % ST == 0

    nc = bacc.Bacc()
    x = nc.dram_tensor("x", [128, NT * 4 * tw], F8, kind="ExternalInput")
    w1t = nc.dram_tensor("w1t", [N_MEM, 4 * 2 * N_MEM], F8, kind="ExternalInput")
    w2t = nc.dram_tensor("w2t", [N_MEM, OUT_LEN], F16, kind="ExternalInput")
    b1 = nc.dram_tensor("b1", [2 * N_MEM, 1], F32, kind="ExternalInput")
    negb2 = nc.dram_tensor("negb2", [OUT_LEN, 1], F32, kind="ExternalInput")
    bscale = nc.dram_tensor("bscale", [N_MEM, 1], F32, kind="ExternalInput")
    out = nc.dram_tensor("out", [OUT_LEN, t_loc], F16, kind="ExternalOutput")

    with ExitStack() as ctx:
        tc = ctx.enter_context(tile.TileContext(nc))
        singles = ctx.enter_context(tc.tile_pool(name="singles", bufs=1))
        xpool = ctx.enter_context(tc.tile_pool(name="xpool", bufs=4))
        hpool = ctx.enter_context(tc.tile_pool(name="hpool", bufs=4))
        opool = ctx.enter_context(tc.tile_pool(name="opool", bufs=4))
        psG = ctx.enter_context(tc.tile_pool(name="psG", bufs=2, space="PSUM"))
        psB = ctx.enter_context(tc.tile_pool(name="psB", bufs=1, space="PSUM"))

        # Persistent full-width recurrence buffers, written at a +1 column
        # offset (a_buf[:, p] = a at input column p-1) so the scan output
        # z[:, p] = u at column p directly.
        a_buf = singles.tile([N_MEM, w_in + 1], F16)
        b_buf = singles.tile([N_MEM, w_in + 1], F16)
        z_buf = singles.tile([N_MEM, w_in], F16)

        # ---- weights / biases (host provides transposed layouts) ----
        w1t_sb = singles.tile([128, 4, 2 * N_MEM], F8)
        nc.sync.dma_start(out=w1t_sb,
                          in_=w1t[:].rearrange("p (k m) -> p k m", k=4))
        w2t_sb = singles.tile([128, 2, N_MEM], F16)
        nc.sync.dma_start(out=w2t_sb,
                          in_=w2t[:].rearrange("p (m j) -> p m j", m=2))
        b1_sb = singles.tile([128, 2, 1], F32)
        nc.sync.dma_start(out=b1_sb, in_=b1[:].rearrange("(m p) o -> p m o", p=128))
        negb2_sb = singles.tile([128, 2, 1], F32)
        nc.sync.dma_start(out=negb2_sb,
                          in_=negb2[:].rearrange("(m p) o -> p m o", p=128))
        bs_sb = singles.tile([128, 1], F32)
        nc.sync.dma_start(out=bs_sb, in_=bscale[:])

        nc.vector.memset(a_buf[:, 0:1], 0.0)
        nc.vector.memset(b_buf[:, 0:1], 0.0)

        # x host layout: [128, tile j (17), k (4), t (512)]
        xr = x[:].rearrange("p (j k t) -> p j k t", j=NT, k=4)
        outr = out[:].rearrange("(m p) t -> p m t", p=128)   # (128, 2, t_loc)

        def gemm1(g0, g1, xt, njj):
            # g0/g1: [128, njj*tw] PSUM; xt: [128, njj, 4, tw] fp8
            for jj in range(njj):
                for k in range(4):
                    nc.tensor.matmul(
                        g0[:, jj * tw:(jj + 1) * tw],
                        lhsT=w1t_sb[:, k, 0:128], rhs=xt[:, jj, k, :],
                        start=(k == 0), stop=(k == 3))
            for jj in range(njj):
                for k in range(4):
                    nc.tensor.matmul(
                        g1[:, jj * tw:(jj + 1) * tw],
                        lhsT=w1t_sb[:, k, 128:256], rhs=xt[:, jj, k, :],
                        start=(k == 0), stop=(k == 3))

        def acts_ab(g0, g1, c0, width):
            left = hpool.tile([128, width], F16)
            right = hpool.tile([128, width], F16)
            nc.scalar.activation(left, g0, SIG, bias=b1_sb[:, 0, :],
                                 scale=1.0 / W1_SCALE)
            nc.scalar.activation(right, g1, SIG, bias=b1_sb[:, 1, :],
                                 scale=1.0 / W1_SCALE)
            nc.vector.tensor_scalar(out=b_buf[:, c0 + 1:c0 + 1 + width],
                                    in0=left, scalar1=-1.0, scalar2=1.0,
                                    op0=MUL, op1=ADD)
            nc.vector.tensor_mul(a_buf[:, c0 + 1:c0 + 1 + width], left, right)

        def phase_c(s):
            # GEMM2 + out activation + store for supertile s (owned cols)
            c0 = halo + s * ST          # z columns
            oc = s * ST                 # output columns
            o0 = psB.tile([128, ST], mybir.dt.float32, tag="o0", bufs=1)
            o1 = psB.tile([128, ST], mybir.dt.float32, tag="o1", bufs=1)
            for jj in range(2):
                zr = z_buf[:, c0 + jj * tw:c0 + (jj + 1) * tw]
                nc.tensor.matmul(o0[:, jj * tw:(jj + 1) * tw],
                                 lhsT=w2t_sb[:, 0, :], rhs=zr,
                                 start=True, stop=True)
            for jj in range(2):
                zr = z_buf[:, c0 + jj * tw:c0 + (jj + 1) * tw]
                nc.tensor.matmul(o1[:, jj * tw:(jj + 1) * tw],
                                 lhsT=w2t_sb[:, 1, :], rhs=zr,
                                 start=True, stop=True)
            ot0 = opool.tile([128, ST], F16)
            ot1 = opool.tile([128, ST], F16)
            nc.scalar.activation(ot0, o0, SIG, bias=negb2_sb[:, 0, :], scale=-1.0)
            nc.scalar.activation(ot1, o1, SIG, bias=negb2_sb[:, 1, :], scale=-1.0)
            nc.sync.dma_start(out=outr[:, 0, oc:oc + ST], in_=ot0)
            nc.sync.dma_start(out=outr[:, 1, oc:oc + ST], in_=ot1)

        # ---- halo prologue (tile 0, 512 cols) ----
        xt0 = xpool.tile([128, 1, 4, tw], F8)
        nc.sync.dma_start(out=xt0, in_=xr[:, 0:1])
        g0h = psG.tile([128, ST], mybir.dt.float32, tag="g0", bufs=1)
        g1h = psG.tile([128, ST], mybir.dt.float32, tag="g1", bufs=1)
        gemm1(g0h[:, 0:tw], g1h[:, 0:tw], xt0, 1)
        acts_ab(g0h[:, 0:tw], g1h[:, 0:tw], 0, tw)
        # Halo b *= bscale (covers cols [0, halo]; col 0 is the memset)
        nc.vector.tensor_scalar(out=b_buf[:, 0:halo + 1],
                                in0=b_buf[:, 0:halo + 1],
                                scalar1=bs_sb[:, 0:1], scalar2=None, op0=MUL)
        nc.vector.tensor_tensor_scan(
            out=z_buf[:, 0:tw], data0=a_buf[:, 0:tw], data1=b_buf[:, 0:tw],
            initial=0.0, op0=MUL, op1=ADD)

        # ---- main loop: supertiles of 1024 over the owned region ----
        # Phase C lags by DELAY supertiles so the in-order PE stream never
        # waits on the serial scan spine.
        DELAY = 2
        for s in range(NST):
            c0 = halo + s * ST
            xt = xpool.tile([128, 2, 4, tw], F8)
            nc.sync.dma_start(out=xt, in_=xr[:, 1 + 2 * s:3 + 2 * s])
            g0 = psG.tile([128, ST], mybir.dt.float32, tag="g0", bufs=1)
            g1 = psG.tile([128, ST], mybir.dt.float32, tag="g1", bufs=1)
            gemm1(g0, g1, xt, 2)
            acts_ab(g0, g1, c0, ST)
            nc.vector.tensor_tensor_scan(
                out=z_buf[:, c0:c0 + ST],
                data0=a_buf[:, c0:c0 + ST],
                data1=b_buf[:, c0:c0 + ST],
                initial=z_buf[:, c0 - 1:c0], op0=MUL, op1=ADD)
            if s - DELAY >= 0:
                phase_c(s - DELAY)

        for s in range(max(0, NST - DELAY), NST):
            phase_c(s)

    nc.finalize()
    return nc


def _make_in_maps(inputs, W1, B1, W2, B2, t_loc=T_LOC, halo=HALO, n_cores=N_CORES):
    inputs = np.asarray(inputs, dtype=np.float32)
    W1 = np.asarray(W1, dtype=np.float32)
    B1 = np.ascontiguousarray(np.asarray(B1, dtype=np.float32))
    W2 = np.asarray(W2, dtype=np.float32)
    B2 = np.asarray(B2, dtype=np.float32)

    x_f8 = np.clip(inputs, -240, 240).astype(F8_NP)
    w1t = np.ascontiguousarray(
        np.clip(W1 * W1_SCALE, -240, 240).T.astype(F8_NP)
        .reshape(4, 128, 2 * N_MEM)
        .transpose(1, 0, 2).reshape(128, 4 * 2 * N_MEM))      # (128, 1024)
    w2t = np.ascontiguousarray(W2.T.astype(F16_NP))           # (128, 256)
    negb2 = np.ascontiguousarray(-B2)                          # (256, 1)

    in_maps = []
    for i in range(n_cores):
        s = i * t_loc
        lo = s - halo
        if lo < 0:
            xs = np.concatenate(
                [np.zeros((INPUT_LEN, -lo), F8_NP), x_f8[:, :s + t_loc]],
                axis=1)
        else:
            xs = x_f8[:, lo:s + t_loc]
        # pack [512, w_in] -> [128 partitions, tile j, k, t]
        xp = np.ascontiguousarray(
            xs.reshape(4, 128, NT, TW).transpose(1, 2, 0, 3)
            .reshape(128, NT * 4 * TW))
        bs = np.full((N_MEM, 1), 0.0 if i == 0 else 1.0, np.float32)
        in_maps.append({
            "x": xp,
            "w1t": w1t, "w2t": w2t, "b1": B1,
            "negb2": negb2, "bscale": bs,
        })
    return in_maps


def _run(inputs, W1, B1, W2, B2, trace=False, **kw):
    from concourse.bass_utils import run_bass_kernel_spmd

    nc = _build_nc()
    in_maps = _make_in_maps(inputs, W1, B1, W2, B2)
    res = run_bass_kernel_spmd(nc, in_maps, list(range(N_CORES)), trace=trace, **kw)
    full = np.concatenate([r["out"] for r in res.results], axis=1)
    return full.astype(np.float32), res


def kernel(inputs, W1, B1, W2, B2):
    full, _ = _run(inputs, W1, B1, W2, B2, trace=False)
    return full.astype(np.float32, copy=False)


# revision 6
# speedup vs baseline: 1.0934x; 1.0934x over previous
"""Deductron kernel for Trainium2, 8 NeuronCores, time-sharded.

Math (matching the reference):
    h = sigmoid(W1 @ x + B1); left, right = h[:128], h[128:]
    a_t = left_t * right_t; b_t = 1 - left_t
    u_0 = 0; u_t = a_{t-1} * u_{t-1} + b_{t-1}   (z[:, t] = u_t)
    out = 1 - sigmoid(W2 @ z + B2) = sigmoid(-(W2 @ z + B2))

Sharding: the 65536-frame time axis is split into 8 chunks of 8192 plus a
512-frame washout halo per core (prod of 512 consecutive a's underflows, so
no cross-core state exchange is needed; core 0 zero-pads and bscale zeroes
the halo b's so its state stays exactly 0).

Perf structure (vs the naive per-512 pipeline):
  - x and W1 are host-quantized to fp8 e4m3 (W1 pre-scaled by 64, descaled in
    the activation's free affine). Halves input DMA; matmul runs at bf16 rate.
  - x is host-packed [128, nt, 4, tw] so one dma_start per supertile moves
    contiguous 4KB-per-partition chunks.
  - 1024-col supertiles: activations read 2 PSUM banks per instruction
    (FD=1024) to amortize the ~312-cycle ScalarE bubble; GEMM2 outputs are
    paired per half so the output activations are also FD=1024.
  - PSUM: psA (GEMM1 g0/g1, 2 bufs x 2 banks) + psB (GEMM2 o0/o1 pairs,
    2 bufs x 2 banks) = 8 banks.
"""

import sys

for _p in ("/opt/trn_rl_repo", "/opt/pypackages"):
    if _p not in sys.path:
        sys.path.append(_p)

import numpy as np
import ml_dtypes

# Problem constants (hardcoded per contract).
INPUT_LEN = 512
N_MEM = 128  # memory dim (recurrence state width) = one partition tile
OUT_LEN = 256
T_TOTAL = 65536
N_CORES = 8
T_LOC = T_TOTAL // N_CORES  # 8192 owned frames per core
HALO = 512                  # washout halo
TW = 512                    # base column tile (one PSUM bank of fp32)
ST = 1024                   # supertile: 2 banks per activation
W_IN = HALO + T_LOC         # per-core input width (8704)
NT = W_IN // TW             # 17 column tiles (tile 0 is pure halo)
NST = T_LOC // ST           # 8 supertiles over the owned region

F16_NP = np.float16
F8_NP = ml_dtypes.float8_e4m3
W1_SCALE = 64.0


def _build_nc(t_loc=T_LOC, halo=HALO, tw=TW):
    import concourse.tile as tile
    from concourse import bacc, mybir
    from contextlib import ExitStack

    F32 = mybir.dt.float32
    F16 = mybir.dt.float16
    F8 = mybir.dt.float8e4
    SIG = mybir.ActivationFunctionType.Sigmoid
    MUL = mybir.AluOpType.mult
    ADD = mybir.AluOpType.add

    w_in = halo + t_loc
    assert w_in % tw == 0 and halo == tw and t_loc % ST == 0

    nc = bacc.Bacc()
    x = nc.dram_tensor("x", [128, NT * 4 * tw], F8, kind="ExternalInput")
    w1t = nc.dram_tensor("w1t", [N_MEM, 4 * 2 * N_MEM], F8, kind="ExternalInput")
    w2t = nc.dram_tensor("w2t", [N_MEM, OUT_LEN], F16, kind="ExternalInput")
    b1 = nc.dram_tensor("b1", [2 * N_MEM, 1], F32, kind="ExternalInput")
    negb2 = nc.dram_tensor("negb2", [OUT_LEN, 1], F32, kind="ExternalInput")
    bscale = nc.dram_tensor("bscale", [N_MEM, 1], F32, kind="ExternalInput")
    out = nc.dram_tensor("out", [OUT_LEN, t_loc], F16, kind="ExternalOutput")

    with ExitStack() as ctx:
        tc = ctx.enter_context(tile.TileContext(nc))
        singles = ctx.enter_context(tc.tile_pool(name="singles", bufs=1))
        xpool = ctx.enter_context(tc.tile_pool(name="xpool", bufs=4))
        hpool = ctx.enter_context(tc.tile_pool(name="hpool", bufs=4))
        opool = ctx.enter_context(tc.tile_pool(name="opool", bufs=4))
        psG = ctx.enter_context(tc.tile_pool(name="psG", bufs=2, space="PSUM"))
        psB = ctx.enter_context(tc.tile_pool(name="psB", bufs=1, space="PSUM"))

        # Persistent full-width recurrence buffers, written at a +1 column
        # offset (a_buf[:, p] = a at input column p-1) so the scan output
        # z[:, p] = u at column p directly.
        a_buf = singles.tile([N_MEM, w_in + 1], F16)
        b_buf = singles.tile([N_MEM, w_in + 1], F16)
        z_buf = singles.tile([N_MEM, w_in], F16)

        # ---- weights / biases (host provides transposed layouts) ----
        w1t_sb = singles.tile([128, 4, 2 * N_MEM], F8)
        nc.sync.dma_start(out=w1t_sb,
                          in_=w1t[:].rearrange("p (k m) -> p k m", k=4))
        w2t_sb = singles.tile([128, 2, N_MEM], F16)
        nc.sync.dma_start(out=w2t_sb,
                          in_=w2t[:].rearrange("p (m j) -> p m j", m=2))
        b1_sb = singles.tile([128, 2, 1], F32)
        nc.sync.dma_start(out=b1_sb, in_=b1[:].rearrange("(m p) o -> p m o", p=128))
        negb2_sb = singles.tile([128, 2, 1], F32)
        nc.sync.dma_start(out=negb2_sb,
                          in_=negb2[:].rearrange("(m p) o -> p m o", p=128))
        bs_sb = singles.tile([128, 1], F32)
        nc.sync.dma_start(out=bs_sb, in_=bscale[:])

        nc.vector.memset(a_buf[:, 0:1], 0.0)
        nc.vector.memset(b_buf[:, 0:1], 0.0)

        # x host layout: [128, tile j (17), k (4), t (512)]
        xr = x[:].rearrange("p (j k t) -> p j k t", j=NT, k=4)
        outr = out[:].rearrange("(m p) t -> p m t", p=128)   # (128, 2, t_loc)

        DR = mybir.MatmulPerfMode.DoubleRow

        def gemm1(g0, g1, xt, njj):
            # g0/g1: [128, njj*tw] PSUM; xt: [128, njj, 4, tw] fp8.
            # DoubleRow: each MM contracts a K=256 pair-block (2 fp8/cell).
            for jj in range(njj):
                for kp in range(2):
                    nc.tensor.matmul(
                        g0[:, jj * tw:(jj + 1) * tw],
                        lhsT=w1t_sb[:, 2 * kp:2 * kp + 2, 0:128],
                        rhs=xt[:, jj, 2 * kp:2 * kp + 2, :],
                        start=(kp == 0), stop=(kp == 1), perf_mode=DR)
            for jj in range(njj):
                for kp in range(2):
                    nc.tensor.matmul(
                        g1[:, jj * tw:(jj + 1) * tw],
                        lhsT=w1t_sb[:, 2 * kp:2 * kp + 2, 128:256],
                        rhs=xt[:, jj, 2 * kp:2 * kp + 2, :],
                        start=(kp == 0), stop=(kp == 1), perf_mode=DR)

        def acts_ab(g0, g1, c0, width):
            left = hpool.tile([128, width], F16)
            right = hpool.tile([128, width], F16)
            nc.scalar.activation(left, g0, SIG, bias=b1_sb[:, 0, :],
                                 scale=1.0 / W1_SCALE)
            nc.scalar.activation(right, g1, SIG, bias=b1_sb[:, 1, :],
                                 scale=1.0 / W1_SCALE)
            nc.vector.tensor_scalar(out=b_buf[:, c0 + 1:c0 + 1 + width],
                                    in0=left, scalar1=-1.0, scalar2=1.0,
                                    op0=MUL, op1=ADD)
            nc.vector.tensor_mul(a_buf[:, c0 + 1:c0 + 1 + width], left, right)

        def phase_c(s):
            # GEMM2 + out activation + store for supertile s (owned cols)
            c0 = halo + s * ST          # z columns
            oc = s * ST                 # output columns
            o0 = psB.tile([128, ST], mybir.dt.float32, tag="o0", bufs=1)
            o1 = psB.tile([128, ST], mybir.dt.float32, tag="o1", bufs=1)
            for jj in range(2):
                zr = z_buf[:, c0 + jj * tw:c0 + (jj + 1) * tw]
                nc.tensor.matmul(o0[:, jj * tw:(jj + 1) * tw],
                                 lhsT=w2t_sb[:, 0, :], rhs=zr,
                                 start=True, stop=True)
            for jj in range(2):
                zr = z_buf[:, c0 + jj * tw:c0 + (jj + 1) * tw]
                nc.tensor.matmul(o1[:, jj * tw:(jj + 1) * tw],
                                 lhsT=w2t_sb[:, 1, :], rhs=zr,
                                 start=True, stop=True)
            ot0 = opool.tile([128, ST], F16)
            ot1 = opool.tile([128, ST], F16)
            nc.scalar.activation(ot0, o0, SIG, bias=negb2_sb[:, 0, :], scale=-1.0)
            nc.scalar.activation(ot1, o1, SIG, bias=negb2_sb[:, 1, :], scale=-1.0)
            nc.sync.dma_start(out=outr[:, 0, oc:oc + ST], in_=ot0)
            nc.sync.dma_start(out=outr[:, 1, oc:oc + ST], in_=ot1)

        # ---- halo prologue (tile 0, 512 cols) ----
        xt0 = xpool.tile([128, 1, 4, tw], F8)
        nc.sync.dma_start(out=xt0, in_=xr[:, 0:1])
        g0h = psG.tile([128, ST], mybir.dt.float32, tag="g0", bufs=1)
        g1h = psG.tile([128, ST], mybir.dt.float32, tag="g1", bufs=1)
        gemm1(g0h[:, 0:tw], g1h[:, 0:tw], xt0, 1)
        acts_ab(g0h[:, 0:tw], g1h[:, 0:tw], 0, tw)
        # Halo b *= bscale (covers cols [0, halo]; col 0 is the memset)
        nc.vector.tensor_scalar(out=b_buf[:, 0:halo + 1],
                                in0=b_buf[:, 0:halo + 1],
                                scalar1=bs_sb[:, 0:1], scalar2=None, op0=MUL)
        nc.vector.tensor_tensor_scan(
            out=z_buf[:, 0:tw], data0=a_buf[:, 0:tw], data1=b_buf[:, 0:tw],
            initial=0.0, op0=MUL, op1=ADD)

        # ---- main loop: supertiles of 1024 over the owned region ----
        # Phase C lags by DELAY supertiles so the in-order PE stream never
        # waits on the serial scan spine.
        DELAY = 2
        for s in range(NST):
            c0 = halo + s * ST
            xt = xpool.tile([128, 2, 4, tw], F8)
            nc.sync.dma_start(out=xt, in_=xr[:, 1 + 2 * s:3 + 2 * s])
            g0 = psG.tile([128, ST], mybir.dt.float32, tag="g0", bufs=1)
            g1 = psG.tile([128, ST], mybir.dt.float32, tag="g1", bufs=1)
            gemm1(g0, g1, xt, 2)
            acts_ab(g0, g1, c0, ST)
            nc.vector.tensor_tensor_scan(
                out=z_buf[:, c0:c0 + ST],
                data0=a_buf[:, c0:c0 + ST],
                data1=b_buf[:, c0:c0 + ST],
                initial=z_buf[:, c0 - 1:c0], op0=MUL, op1=ADD)
            if s - DELAY >= 0:
                phase_c(s - DELAY)

        for s in range(max(0, NST - DELAY), NST):
            phase_c(s)

    nc.finalize()
    return nc


def _make_in_maps(inputs, W1, B1, W2, B2, t_loc=T_LOC, halo=HALO, n_cores=N_CORES):
    inputs = np.asarray(inputs, dtype=np.float32)
    W1 = np.asarray(W1, dtype=np.float32)
    B1 = np.ascontiguousarray(np.asarray(B1, dtype=np.float32))
    W2 = np.asarray(W2, dtype=np.float32)
    B2 = np.asarray(B2, dtype=np.float32)

    x_f8 = np.clip(inputs, -240, 240).astype(F8_NP)
    w1t = np.ascontiguousarray(
        np.clip(W1 * W1_SCALE, -240, 240).T.astype(F8_NP)
        .reshape(4, 128, 2 * N_MEM)
        .transpose(1, 0, 2).reshape(128, 4 * 2 * N_MEM))      # (128, 1024)
    w2t = np.ascontiguousarray(W2.T.astype(F16_NP))           # (128, 256)
    negb2 = np.ascontiguousarray(-B2)                          # (256, 1)

    in_maps = []
    for i in range(n_cores):
        s = i * t_loc
        lo = s - halo
        if lo < 0:
            xs = np.concatenate(
                [np.zeros((INPUT_LEN, -lo), F8_NP), x_f8[:, :s + t_loc]],
                axis=1)
        else:
            xs = x_f8[:, lo:s + t_loc]
        # pack [512, w_in] -> [128 partitions, tile j, k, t]
        xp = np.ascontiguousarray(
            xs.reshape(4, 128, NT, TW).transpose(1, 2, 0, 3)
            .reshape(128, NT * 4 * TW))
        bs = np.full((N_MEM, 1), 0.0 if i == 0 else 1.0, np.float32)
        in_maps.append({
            "x": xp,
            "w1t": w1t, "w2t": w2t, "b1": B1,
            "negb2": negb2, "bscale": bs,
        })
    return in_maps


def _run(inputs, W1, B1, W2, B2, trace=False, **kw):
    from concourse.bass_utils import run_bass_kernel_spmd

    nc = _build_nc()
    in_maps = _make_in_maps(inputs, W1, B1, W2, B2)
    res = run_bass_kernel_spmd(nc, in_maps, list(range(N_CORES)), trace=trace, **kw)
    full = np.concatenate([r["out"] for r in res.results], axis=1)
    return full.astype(np.float32), res


def kernel(inputs, W1, B1, W2, B2):
    full, _ = _run(inputs, W1, B1, W2, B2, trace=False)
    return full.astype(np.float32, copy=False)


# revision 7
# speedup vs baseline: 1.1036x; 1.0093x over previous
"""Deductron kernel for Trainium2, 8 NeuronCores, time-sharded.

Math (matching the reference):
    h = sigmoid(W1 @ x + B1); left, right = h[:128], h[128:]
    a_t = left_t * right_t; b_t = 1 - left_t
    u_0 = 0; u_t = a_{t-1} * u_{t-1} + b_{t-1}   (z[:, t] = u_t)
    out = 1 - sigmoid(W2 @ z + B2) = sigmoid(-(W2 @ z + B2))

Sharding: the 65536-frame time axis is split into 8 chunks of 8192 plus a
512-frame washout halo per core (prod of 512 consecutive a's underflows, so
no cross-core state exchange is needed; core 0 zero-pads and bscale zeroes
the halo b's so its state stays exactly 0).

Perf structure (vs the naive per-512 pipeline):
  - x and W1 are host-quantized to fp8 e4m3 (W1 pre-scaled by 64, descaled in
    the activation's free affine). Halves input DMA; matmul runs at bf16 rate.
  - x is host-packed [128, nt, 4, tw] so one dma_start per supertile moves
    contiguous 4KB-per-partition chunks.
  - 1024-col supertiles: activations read 2 PSUM banks per instruction
    (FD=1024) to amortize the ~312-cycle ScalarE bubble; GEMM2 outputs are
    paired per half so the output activations are also FD=1024.
  - PSUM: psA (GEMM1 g0/g1, 2 bufs x 2 banks) + psB (GEMM2 o0/o1 pairs,
    2 bufs x 2 banks) = 8 banks.
"""

import sys

for _p in ("/opt/trn_rl_repo", "/opt/pypackages"):
    if _p not in sys.path:
        sys.path.append(_p)

import numpy as np
import ml_dtypes

# Problem constants (hardcoded per contract).
INPUT_LEN = 512
N_MEM = 128  # memory dim (recurrence state width) = one partition tile
OUT_LEN = 256
T_TOTAL = 65536
N_CORES = 8
T_LOC = T_TOTAL // N_CORES  # 8192 owned frames per core
HALO = 512                  # washout halo
TW = 512                    # base column tile (one PSUM bank of fp32)
ST = 1024                   # supertile: 2 banks per activation
W_IN = HALO + T_LOC         # per-core input width (8704)
NT = W_IN // TW             # 17 column tiles (tile 0 is pure halo)
NST = T_LOC // ST           # 8 supertiles over the owned region

F16_NP = np.float16
F8_NP = ml_dtypes.float8_e4m3
W1_SCALE = 64.0


def _build_nc(t_loc=T_LOC, halo=HALO, tw=TW):
    import concourse.tile as tile
    from concourse import bacc, mybir
    from contextlib import ExitStack

    F32 = mybir.dt.float32
    F16 = mybir.dt.float16
    F8 = mybir.dt.float8e4
    SIG = mybir.ActivationFunctionType.Sigmoid
    MUL = mybir.AluOpType.mult
    ADD = mybir.AluOpType.add

    w_in = halo + t_loc
    assert w_in % tw == 0 and halo == tw and t_loc % ST == 0

    nc = bacc.Bacc()
    x = nc.dram_tensor("x", [128, NT * 4 * tw], F8, kind="ExternalInput")
    w1t = nc.dram_tensor("w1t", [N_MEM, 4 * 2 * N_MEM], F8, kind="ExternalInput")
    w2t = nc.dram_tensor("w2t", [N_MEM, OUT_LEN], F16, kind="ExternalInput")
    b1 = nc.dram_tensor("b1", [2 * N_MEM, 1], F32, kind="ExternalInput")
    negb2 = nc.dram_tensor("negb2", [OUT_LEN, 1], F32, kind="ExternalInput")
    bscale = nc.dram_tensor("bscale", [N_MEM, 1], F32, kind="ExternalInput")
    out = nc.dram_tensor("out", [OUT_LEN, t_loc], F16, kind="ExternalOutput")

    with ExitStack() as ctx:
        tc = ctx.enter_context(tile.TileContext(nc))
        singles = ctx.enter_context(tc.tile_pool(name="singles", bufs=1))
        xpool = ctx.enter_context(tc.tile_pool(name="xpool", bufs=4))
        hpool = ctx.enter_context(tc.tile_pool(name="hpool", bufs=4))
        opool = ctx.enter_context(tc.tile_pool(name="opool", bufs=4))
        psG = ctx.enter_context(tc.tile_pool(name="psG", bufs=2, space="PSUM"))
        psB = ctx.enter_context(tc.tile_pool(name="psB", bufs=1, space="PSUM"))

        # Persistent full-width recurrence buffers, written at a +1 column
        # offset (a_buf[:, p] = a at input column p-1) so the scan output
        # z[:, p] = u at column p directly.
        a_buf = singles.tile([N_MEM, w_in + 1], F16)
        b_buf = singles.tile([N_MEM, w_in + 1], F16)
        z_buf = singles.tile([N_MEM, w_in], F16)

        # Pull the sigmoid ACT_TABLE_LOAD (~2.7us) off the critical path:
        # a dependency-free dummy activation issued before any real work.
        dummy = singles.tile([128, 1], F32)
        nc.vector.memset(dummy, 0.0)
        nc.scalar.activation(dummy, dummy, SIG)

        # ---- weights / biases (host provides transposed layouts) ----
        # Order matters for startup latency: w1t and b1 gate the first
        # GEMM1+activation; everything else is needed later.
        w1t_sb = singles.tile([128, 4, 2 * N_MEM], F8)
        nc.sync.dma_start(out=w1t_sb,
                          in_=w1t[:].rearrange("p (k m) -> p k m", k=4))
        b1_sb = singles.tile([128, 2, 1], F32)
        nc.sync.dma_start(out=b1_sb, in_=b1[:].rearrange("(m p) o -> p m o", p=128))
        bs_sb = singles.tile([128, 1], F32)
        nc.sync.dma_start(out=bs_sb, in_=bscale[:])
        w2t_sb = singles.tile([128, 2, N_MEM], F16)
        nc.sync.dma_start(out=w2t_sb,
                          in_=w2t[:].rearrange("p (m j) -> p m j", m=2))
        negb2_sb = singles.tile([128, 2, 1], F32)
        nc.sync.dma_start(out=negb2_sb,
                          in_=negb2[:].rearrange("(m p) o -> p m o", p=128))

        nc.vector.memset(a_buf[:, 0:1], 0.0)
        nc.vector.memset(b_buf[:, 0:1], 0.0)

        # x host layout: [128, tile j (17), k (4), t (512)]
        xr = x[:].rearrange("p (j k t) -> p j k t", j=NT, k=4)
        outr = out[:].rearrange("(m p) t -> p m t", p=128)   # (128, 2, t_loc)

        DR = mybir.MatmulPerfMode.DoubleRow

        def gemm1(g0, g1, xt, njj):
            # g0/g1: [128, njj*tw] PSUM; xt: [128, njj, 4, tw] fp8.
            # DoubleRow: each MM contracts a K=256 pair-block (2 fp8/cell).
            for jj in range(njj):
                for kp in range(2):
                    nc.tensor.matmul(
                        g0[:, jj * tw:(jj + 1) * tw],
                        lhsT=w1t_sb[:, 2 * kp:2 * kp + 2, 0:128],
                        rhs=xt[:, jj, 2 * kp:2 * kp + 2, :],
                        start=(kp == 0), stop=(kp == 1), perf_mode=DR)
            for jj in range(njj):
                for kp in range(2):
                    nc.tensor.matmul(
                        g1[:, jj * tw:(jj + 1) * tw],
                        lhsT=w1t_sb[:, 2 * kp:2 * kp + 2, 128:256],
                        rhs=xt[:, jj, 2 * kp:2 * kp + 2, :],
                        start=(kp == 0), stop=(kp == 1), perf_mode=DR)

        def acts_ab(g0, g1, c0, width):
            left = hpool.tile([128, width], F16)
            right = hpool.tile([128, width], F16)
            nc.scalar.activation(left, g0, SIG, bias=b1_sb[:, 0, :],
                                 scale=1.0 / W1_SCALE)
            nc.scalar.activation(right, g1, SIG, bias=b1_sb[:, 1, :],
                                 scale=1.0 / W1_SCALE)
            nc.vector.tensor_scalar(out=b_buf[:, c0 + 1:c0 + 1 + width],
                                    in0=left, scalar1=-1.0, scalar2=1.0,
                                    op0=MUL, op1=ADD)
            nc.vector.tensor_mul(a_buf[:, c0 + 1:c0 + 1 + width], left, right)

        def phase_c(s):
            # GEMM2 + out activation + store for supertile s (owned cols)
            c0 = halo + s * ST          # z columns
            oc = s * ST                 # output columns
            o0 = psB.tile([128, ST], mybir.dt.float32, tag="o0", bufs=1)
            o1 = psB.tile([128, ST], mybir.dt.float32, tag="o1", bufs=1)
            for jj in range(2):
                zr = z_buf[:, c0 + jj * tw:c0 + (jj + 1) * tw]
                nc.tensor.matmul(o0[:, jj * tw:(jj + 1) * tw],
                                 lhsT=w2t_sb[:, 0, :], rhs=zr,
                                 start=True, stop=True)
            for jj in range(2):
                zr = z_buf[:, c0 + jj * tw:c0 + (jj + 1) * tw]
                nc.tensor.matmul(o1[:, jj * tw:(jj + 1) * tw],
                                 lhsT=w2t_sb[:, 1, :], rhs=zr,
                                 start=True, stop=True)
            ot0 = opool.tile([128, ST], F16)
            ot1 = opool.tile([128, ST], F16)
            nc.scalar.activation(ot0, o0, SIG, bias=negb2_sb[:, 0, :], scale=-1.0)
            nc.scalar.activation(ot1, o1, SIG, bias=negb2_sb[:, 1, :], scale=-1.0)
            nc.sync.dma_start(out=outr[:, 0, oc:oc + ST], in_=ot0)
            nc.sync.dma_start(out=outr[:, 1, oc:oc + ST], in_=ot1)

        # ---- halo prologue (tile 0, 512 cols) ----
        xt0 = xpool.tile([128, 1, 4, tw], F8)
        nc.sync.dma_start(out=xt0, in_=xr[:, 0:1])
        g0h = psG.tile([128, ST], mybir.dt.float32, tag="g0", bufs=1)
        g1h = psG.tile([128, ST], mybir.dt.float32, tag="g1", bufs=1)
        gemm1(g0h[:, 0:tw], g1h[:, 0:tw], xt0, 1)
        acts_ab(g0h[:, 0:tw], g1h[:, 0:tw], 0, tw)
        # Halo b *= bscale (covers cols [0, halo]; col 0 is the memset)
        nc.vector.tensor_scalar(out=b_buf[:, 0:halo + 1],
                                in0=b_buf[:, 0:halo + 1],
                                scalar1=bs_sb[:, 0:1], scalar2=None, op0=MUL)
        nc.vector.tensor_tensor_scan(
            out=z_buf[:, 0:tw], data0=a_buf[:, 0:tw], data1=b_buf[:, 0:tw],
            initial=0.0, op0=MUL, op1=ADD)

        # ---- main loop: supertiles of 1024 over the owned region ----
        # Phase C lags by DELAY supertiles so the in-order PE stream never
        # waits on the serial scan spine.
        DELAY = 2
        for s in range(NST):
            c0 = halo + s * ST
            xt = xpool.tile([128, 2, 4, tw], F8)
            nc.sync.dma_start(out=xt, in_=xr[:, 1 + 2 * s:3 + 2 * s])
            g0 = psG.tile([128, ST], mybir.dt.float32, tag="g0", bufs=1)
            g1 = psG.tile([128, ST], mybir.dt.float32, tag="g1", bufs=1)
            gemm1(g0, g1, xt, 2)
            acts_ab(g0, g1, c0, ST)
            nc.vector.tensor_tensor_scan(
                out=z_buf[:, c0:c0 + ST],
                data0=a_buf[:, c0:c0 + ST],
                data1=b_buf[:, c0:c0 + ST],
                initial=z_buf[:, c0 - 1:c0], op0=MUL, op1=ADD)
            if s - DELAY >= 0:
                phase_c(s - DELAY)

        for s in range(max(0, NST - DELAY), NST):
            phase_c(s)

    nc.finalize()
    return nc


def _make_in_maps(inputs, W1, B1, W2, B2, t_loc=T_LOC, halo=HALO, n_cores=N_CORES):
    inputs = np.asarray(inputs, dtype=np.float32)
    W1 = np.asarray(W1, dtype=np.float32)
    B1 = np.ascontiguousarray(np.asarray(B1, dtype=np.float32))
    W2 = np.asarray(W2, dtype=np.float32)
    B2 = np.asarray(B2, dtype=np.float32)

    x_f8 = np.clip(inputs, -240, 240).astype(F8_NP)
    w1t = np.ascontiguousarray(
        np.clip(W1 * W1_SCALE, -240, 240).T.astype(F8_NP)
        .reshape(4, 128, 2 * N_MEM)
        .transpose(1, 0, 2).reshape(128, 4 * 2 * N_MEM))      # (128, 1024)
    w2t = np.ascontiguousarray(W2.T.astype(F16_NP))           # (128, 256)
    negb2 = np.ascontiguousarray(-B2)                          # (256, 1)

    in_maps = []
    for i in range(n_cores):
        s = i * t_loc
        lo = s - halo
        if lo < 0:
            xs = np.concatenate(
                [np.zeros((INPUT_LEN, -lo), F8_NP), x_f8[:, :s + t_loc]],
                axis=1)
        else:
            xs = x_f8[:, lo:s + t_loc]
        # pack [512, w_in] -> [128 partitions, tile j, k, t]
        xp = np.ascontiguousarray(
            xs.reshape(4, 128, NT, TW).transpose(1, 2, 0, 3)
            .reshape(128, NT * 4 * TW))
        bs = np.full((N_MEM, 1), 0.0 if i == 0 else 1.0, np.float32)
        in_maps.append({
            "x": xp,
            "w1t": w1t, "w2t": w2t, "b1": B1,
            "negb2": negb2, "bscale": bs,
        })
    return in_maps


def _run(inputs, W1, B1, W2, B2, trace=False, **kw):
    from concourse.bass_utils import run_bass_kernel_spmd

    nc = _build_nc()
    in_maps = _make_in_maps(inputs, W1, B1, W2, B2)
    res = run_bass_kernel_spmd(nc, in_maps, list(range(N_CORES)), trace=trace, **kw)
    full = np.concatenate([r["out"] for r in res.results], axis=1)
    return full.astype(np.float32), res


def kernel(inputs, W1, B1, W2, B2):
    full, _ = _run(inputs, W1, B1, W2, B2, trace=False)
    return full.astype(np.float32, copy=False)
